# revision 13
# baseline (speedup 1.0000x reference)
"""LIF spiking-neuron kernel for Trainium2 (Bass/Tile), 8-core SPMD.

Problem: x [B=32, T=8, C=128, H=32, W=32] f32.  Per (b,c,h,w) neuron,
sequential over T:
    mem = mem*TAU + x_t;  spike = (mem - 1 > 0);  mem = 0 if spike
TAU = 0.5, THRESH = 1.0.

Sharding: batch dim B=32 split 4-per-core across 8 NeuronCores; the
recurrence is along T only, so there is no communication.

Bit-exact algorithm vs the fp32 reference:
  TAU = 0.5 is a power of two, so rescale the state M_t = 2^t * m_t.
  The decay becomes a pure add:  M_t = M_{t-1} + y_t with y_t = 2^t*x_t
  (prescaled exactly on the HOST - power-of-2 scaling commutes with fp
  rounding, so every M_t is bit-exactly 2^t * m_t).
  spike_t = (M_t > 2^t)  <=>  reference's (m_t - 1 > 0).

Engine assignment (4 independent chains, one per local batch, tiles
[C=128 partitions, H*W=1024]):
  DMA:  the accumulate M += y_t rides the input DMA itself
        (gpsimd SWDGE accum_op=add, verified bit-exact on HW) - zero
        compute-engine cost for the recurrence add.
  DVE:  reset only: M' = (M <= 2^t)*M (fused stt is_le+mult, maskless)
  ACT:  spike u8 = Sign(M - 2^t) -> u8; the saturating fp32->u8 cast
        maps Sign's -1 to 0 (verified on HW) - single pass, off the
        recurrence critical path
  SP :  t=0 input DMAs + u8 output DMAs (HWDGE queues; outputs here so
        they never head-of-line block the gpsimd accumulate stream)

HBM traffic: 16 MiB in + 4 MiB u8 out per core (spikes cast to f32 on
the host).
"""

import numpy as np

from concourse import bacc, bass, mybir, tile
from concourse.alu_op_type import AluOpType
from concourse.bass_utils import run_bass_kernel_spmd

# Full-problem shape (hardcoded per harness contract).
B, T, C, H, W = 32, 8, 128, 32, 32
N_CORES = 8
B_LOC = B // N_CORES          # 4 batches per core
F = H * W                     # 1024 free elements per chain tile
FW = B_LOC * F                # 4096 free elements per fat output tile
FP32 = mybir.dt.float32
U8 = mybir.dt.uint8

_NC_CACHE = {}


def _emit(tc, y_d, o_d):
    nc = tc.nc

    with (
        tc.tile_pool(name="sp", bufs=3) as sp,
        tc.tile_pool(name="mp", bufs=1) as mp,
    ):
        # ping-pong membrane state per chain
        ms = [
            [mp.tile([C, F], FP32, name=f"m{g}_{i}") for i in range(2)]
            for g in range(B_LOC)
        ]
        for t in range(T):
            th = float(2.0**t)
            s_fat = sp.tile([C, FW], U8)
            for g in range(B_LOC):
                cur = ms[g][t % 2]
                y_gt = y_d[g, t].rearrange("c h w -> c (h w)")
                if t == 0:
                    eng = nc.sync if g % 2 == 0 else nc.scalar
                    eng.dma_start(out=cur, in_=y_gt)
                else:
                    # recurrence add in-flight: M += y_t
                    nc.gpsimd.dma_start(
                        out=cur, in_=y_gt, accum_op=AluOpType.add
                    )
                # spike u8 in one ACT pass (saturating cast: Sign's -1 -> 0)
                nc.scalar.activation(
                    s_fat[:, g * F : (g + 1) * F],
                    cur,
                    mybir.ActivationFunctionType.Sign,
                    bias=-th,
                )
                if t < T - 1:
                    # fused maskless reset into the other ping-pong tile
                    nc.vector.scalar_tensor_tensor(
                        ms[g][(t + 1) % 2], cur, th, cur,
                        AluOpType.is_le, AluOpType.mult,
                    )
            # u8 spikes out via HWDGE (SP/ACT queues are otherwise idle)
            out_eng = nc.sync if t % 2 == 0 else nc.scalar
            out_eng.dma_start(
                out=o_d[:, t].rearrange("b c h w -> c b (h w)"),
                in_=s_fat.rearrange("c (b f) -> c b f", b=B_LOC),
            )


def build_nc():
    """Build + compile the per-core Bass program (cached)."""
    if "nc" in _NC_CACHE:
        return _NC_CACHE["nc"]
    nc = bacc.Bacc(
        "TRN2",
        target_bir_lowering=False,
        debug=False,
        enable_asserts=False,
        num_devices=N_CORES,
    )
    y_d = nc.dram_tensor("y", [B_LOC, T, C, H, W], FP32, kind="ExternalInput").ap()
    o_d = nc.dram_tensor("out", [B_LOC, T, C, H, W], U8, kind="ExternalOutput").ap()
    # register -2^t bias constants (memset in the preamble, like builtin consts)
    for t in range(T):
        v = -float(2.0**t)
        th_t = nc.alloc_sbuf_tensor(f"const-float32-{v}", [C, 1], FP32)
        nc.gpsimd.memset(th_t.ap(), v)
        nc.const_aps.aps[(FP32, v)] = th_t.ap()
    with tile.TileContext(nc) as tc:
        _emit(tc, y_d, o_d)
    nc.compile()
    _NC_CACHE["nc"] = nc
    return nc


_POW2 = (2.0 ** np.arange(T, dtype=np.float32))[None, :, None, None, None]


def make_in_maps(x: np.ndarray) -> list[dict[str, np.ndarray]]:
    assert x.shape == (B, T, C, H, W) and x.dtype == np.float32, (x.shape, x.dtype)
    y = x * _POW2  # exact power-of-2 prescale on the host
    return [
        {"y": np.ascontiguousarray(y[i * B_LOC : (i + 1) * B_LOC])}
        for i in range(N_CORES)
    ]


def kernel(x: np.ndarray) -> np.ndarray:
    x = np.asarray(x, dtype=np.float32)
    nc = build_nc()
    res = run_bass_kernel_spmd(nc, make_in_maps(x), list(range(N_CORES)))
    return np.concatenate([r["out"] for r in res.results], axis=0).astype(np.float32)


# revision 16
# speedup vs baseline: 1.0961x; 1.0961x over previous
"""LIF spiking-neuron kernel for Trainium2 (Bass/Tile), 8-core SPMD.

Problem: x [B=32, T=8, C=128, H=32, W=32] f32.  Per (b,c,h,w) neuron,
sequential over T:
    mem = mem*TAU + x_t;  spike = (mem - 1 > 0);  mem = 0 if spike
TAU = 0.5, THRESH = 1.0.

Sharding: batch dim B=32 split 4-per-core across 8 NeuronCores; the
recurrence is along T only, so there is no communication.

Per-core algorithm (bit-exact vs the fp32 reference):
  TAU = 0.5 is a power of two, so rescale the state M_t = 2^t * m_t.
  The decay becomes a pure add:  M_t = M_{t-1} + 2^t * x_t  (the 2^t
  prescale of x is exact in fp32, and power-of-2 scaling commutes with
  fp rounding, so every M_t is bit-exactly 2^t * m_t).
  spike_t = (M_t > 2^t)  <=>  (m_t > 1)  <=>  reference's (m_t - 1 > 0).

HBM traffic: input 16 MiB f32/core; spikes leave the device as uint8
(4 MiB/core instead of 16 MiB f32) and are cast to f32 on the host.

Structure: ONE fat tile [C=128 partitions, B_loc*H*W=4096] per step t
(all 4 local batches share the op/DMA; threshold is uniform).  Only 16
DVE ops, 8 ACT ops, 8+8 DMAs total -> minimal per-instruction +
semaphore overhead.  The membrane state ping-pongs between tiles so the
ACT spike compare (Sign(M - 2^t) -> u8, saturating cast maps -1 to 0;
verified on HW) reads M_t concurrently with the DVE reset:
  DVE:  t=0   M = (x_0 <= 1)*x_0          (fused stt is_le+mult)
        t>=1  M' = x_t*2^t + M            (fused prescale+acc stt)
        t<=6  M'' = (M' <= 2^t)*M'        (fused maskless reset stt)
  ACT:  s_t u8 = Sign(M' - 2^t)           (off the recurrence path)
  DMA:  input 2 MiB/t alternating SP/ACT HWDGE queues; output 512 KiB/t
        u8 on the SP queue.
"""

import numpy as np

from concourse import bacc, bass, mybir, tile
from concourse.alu_op_type import AluOpType
from concourse.bass_utils import run_bass_kernel_spmd

# Full-problem shape (hardcoded per harness contract).
B, T, C, H, W = 32, 8, 128, 32, 32
N_CORES = 8
B_LOC = B // N_CORES          # 4 batches per core
F = H * W                     # 1024 free elements per batch
FW = B_LOC * F                # 4096 free elements per fat tile
FP32 = mybir.dt.float32
U8 = mybir.dt.uint8

_NC_CACHE = {}


def _emit(tc, x_d, o_d):
    nc = tc.nc

    def dram(ap, t):
        return ap[:, t].rearrange("b c h w -> c b (h w)")

    with (
        tc.tile_pool(name="xp", bufs=8) as xp,
        tc.tile_pool(name="sp", bufs=3) as sp,
        tc.tile_pool(name="mp", bufs=1) as mp,
    ):
        ms = [mp.tile([C, FW], FP32, name=f"m{i}") for i in range(3)]
        m_prev = None
        for t in range(T):
            th = float(2.0**t)
            xt = xp.tile([C, FW], FP32)
            if t == 0:
                # chunked per-batch DMA + reset: compute starts on the first
                # 512 KiB instead of waiting for the full 2 MiB transfer
                for b in range(B_LOC):
                    eng = nc.sync if b % 2 == 0 else nc.scalar
                    eng.dma_start(
                        out=xt[:, b * F : (b + 1) * F],
                        in_=x_d[b, 0].rearrange("c h w -> c (h w)"),
                    )
                    # fused copy+reset: M = (x_0 <= 1) * x_0
                    nc.vector.scalar_tensor_tensor(
                        ms[0][:, b * F : (b + 1) * F],
                        xt[:, b * F : (b + 1) * F], 1.0,
                        xt[:, b * F : (b + 1) * F],
                        AluOpType.is_le, AluOpType.mult,
                    )
                m_cur = ms[0]
                pre = xt
            else:
                dma_eng = nc.sync if t % 2 == 0 else nc.scalar
                dma_eng.dma_start(
                    out=xt.rearrange("c (b f) -> c b f", b=B_LOC),
                    in_=dram(x_d, t),
                )
                m_cur = ms[t % 3]
                pre = m_cur
            halves = 2 if t == T - 1 else 1
            HS = FW // halves
            for h in range(halves):
                hs = slice(h * HS, (h + 1) * HS)
                if t > 0:
                    # fused prescale+accumulate: M' = (x_t * 2^t) + M
                    nc.vector.scalar_tensor_tensor(
                        m_cur[:, hs], xt[:, hs], th, m_prev[:, hs],
                        AluOpType.mult, AluOpType.add,
                    )
                # spike u8 in one ACT pass: Sign(M*2^-t - 1) -> u8; the
                # power-of-2 scale is exact and the saturating cast maps
                # Sign's -1 to 0 (verified on HW)
                s = sp.tile([C, HS], U8, name=f"s{halves}")
                nc.scalar.activation(
                    s, pre[:, hs], mybir.ActivationFunctionType.Sign,
                    bias=-1.0, scale=1.0 / th,
                )
                # u8 spikes out via gpsimd SWDGE: its own DMA queue, so
                # output triggers never block the input prefetch on SP/ACT
                nb = B_LOC // halves
                nc.gpsimd.dma_start(
                    out=o_d[h * nb : (h + 1) * nb, t].rearrange(
                        "b c h w -> c b (h w)"
                    ),
                    in_=s.rearrange("c (b f) -> c b f", b=nb),
                )
            if t < T - 1:
                # fused maskless reset into the next ping-pong tile
                m_rst = ms[(t + 1) % 3]
                nc.vector.scalar_tensor_tensor(
                    m_rst, m_cur, th, m_cur, AluOpType.is_le, AluOpType.mult
                )
                m_prev = m_rst


def build_nc():
    """Build + compile the per-core Bass program (cached)."""
    if "nc" in _NC_CACHE:
        return _NC_CACHE["nc"]
    nc = bacc.Bacc(
        "TRN2",
        target_bir_lowering=False,
        debug=False,
        enable_asserts=False,
        num_devices=N_CORES,
    )
    x_d = nc.dram_tensor("x", [B_LOC, T, C, H, W], FP32, kind="ExternalInput").ap()
    o_d = nc.dram_tensor("out", [B_LOC, T, C, H, W], U8, kind="ExternalOutput").ap()
    # register the -1.0 bias constant (memset in the preamble)
    th_t = nc.alloc_sbuf_tensor("const-float32--1.0", [C, 1], FP32)
    nc.gpsimd.memset(th_t.ap(), -1.0)
    nc.const_aps.aps[(FP32, -1.0)] = th_t.ap()
    with tile.TileContext(nc) as tc:
        _emit(tc, x_d, o_d)
    nc.compile()
    _NC_CACHE["nc"] = nc
    return nc


def make_in_maps(x: np.ndarray) -> list[dict[str, np.ndarray]]:
    assert x.shape == (B, T, C, H, W) and x.dtype == np.float32, (x.shape, x.dtype)
    return [
        {"x": np.ascontiguousarray(x[i * B_LOC : (i + 1) * B_LOC])}
        for i in range(N_CORES)
    ]


def kernel(x: np.ndarray) -> np.ndarray:
    x = np.asarray(x, dtype=np.float32)
    nc = build_nc()
    res = run_bass_kernel_spmd(nc, make_in_maps(x), list(range(N_CORES)))
    return np.concatenate([r["out"] for r in res.results], axis=0).astype(np.float32)
